# revision 14
# baseline (speedup 1.0000x reference)
"""Trainium2 Bass kernel for the DH-LIF node single-step forward.

Math: the mask is one-hot over the branch dim NB, so

    spike = ( x @ (W + M).T + b >= 1/(1-beta) ),
    M[h,i] = oma[h, idx[h,i]],  oma[h,k] = 0.5*(1 - sigmoid(tau_n[h,k]))

The 2-bit branch index idx is re-encoded on the host (losslessly) as two
{0,1} bit planes t1 = idx&1, t2 = idx>>1 (fp8, h-major).  On device, M is
built per h-tile in the bilinear-bit basis with per-partition coefficients

    M = (A + B*t1) + (C + D*t1)*t2        (A..D per-h from sigmoid(tau_n))

as two DVE tensor_scalar ops (4x mode, dual per-partition scalars) and two
tensor_tensor ops (2x); multiplies by {0,1} are exact in fp16, so precision
matches a single-rounding f32 build.  Everything is scaled by 256 to keep
fp16 away from subnormals.  The [h,i] result is flipped to i-major with PE
transposes; the PSUM->SBUF readback is fused with the +W pass (gpsimd
tensor_tensor against host-pre-transposed fp16 W), giving the matmul lhsT.
x ships transposed fp8 (spikes exact in fp8); one fp16 x fp8 matmul pass
accumulates out[h,b] over 32 k-chunks; PSUM is thresholded against
256/(1-beta) - 256*b per partition.  Dummy PE matmuls pad pipeline gaps so
the tensor engine's activity-gated clock stays at full rate.

Sharding: hidden dim split across 8 cores (h_loc = 256); x replicated.
Host does layout/dtype prep (bit-plane extraction, transposes, packing)
and the final gather/cast.
"""

import numpy as np
import ml_dtypes

B, I, H, NB = 512, 4096, 2048, 4
NCORES = 8
H_LOC = H // NCORES          # 256
N_HT = H_LOC // 128          # 2 h-tiles of 128
N_CHUNK = I // 128           # 32 matmul k-chunks
NQ = 4                       # DMA/compute super-chunks
CPQ = N_CHUNK // NQ          # 8 k-chunks per super-chunk
IQ = CPQ * 128               # 1024 i per super-chunk
SCALE = 256.0                # fp16 subnormal guard (power of 2, lossless)
N_WARM = 20                  # initial PE warmup matmuls
N_FILL = 12                  # dummy matmuls per (q,ht) to keep PE clock hot

TRACE = False
LAST_RESULTS = None
_CACHED = {}


def _build_bass():
    import concourse.bacc as bacc
    import concourse.mybir as mybir
    from concourse.tile import TileContext
    from concourse.masks import make_identity

    f32 = mybir.dt.float32
    f16 = mybir.dt.float16
    fp8 = mybir.dt.float8e4
    AF = mybir.ActivationFunctionType
    ALU = mybir.AluOpType

    nc = bacc.Bacc("TRN2", target_bir_lowering=False, debug=False)

    # h-major bit planes: [128, (ht, i)]
    t1_d = nc.dram_tensor("t1", [128, N_HT * I], fp8, kind="ExternalInput")
    t2_d = nc.dram_tensor("t2", [128, N_HT * I], fp8, kind="ExternalInput")
    # i-major transposed W * SCALE: [128, (chunk, ht, h)]
    w_d = nc.dram_tensor("w", [128, N_CHUNK * H_LOC], f16, kind="ExternalInput")
    # i-major x: [128, (chunk, b)]
    x_d = nc.dram_tensor("x", [128, N_CHUNK * B], fp8, kind="ExternalInput")
    par_d = nc.dram_tensor("par", [128, 6 * N_HT], f32, kind="ExternalInput")
    out_d = nc.dram_tensor("out", [H_LOC, B], fp8, kind="ExternalOutput")

    t1_v = t1_d.rearrange("p (t i) -> p t i", t=N_HT)
    t2_v = t2_d.rearrange("p (t i) -> p t i", t=N_HT)

    with TileContext(nc) as tc:
        with (
            tc.tile_pool(name="const", bufs=1) as cpool,
            tc.tile_pool(name="strm", bufs=1) as spool,
            tc.tile_pool(name="po", bufs=2, space="PSUM") as psum_o,
            tc.tile_pool(name="pt", bufs=3, space="PSUM") as psum_t,
            tc.tile_pool(name="pw", bufs=1, space="PSUM") as psum_w,
        ):
            # ---- bulk DMAs, super-chunk interleaved, issue first ----
            par = cpool.tile([128, 6 * N_HT], f32)
            nc.sync.dma_start(par[:], par_d[:, :])
            t1_8, t2_8, w16, x8 = [], [], [], []
            for q in range(NQ):
                isl = slice(q * IQ, (q + 1) * IQ)
                wsl = slice(q * CPQ * H_LOC, (q + 1) * CPQ * H_LOC)
                xsl = slice(q * CPQ * B, (q + 1) * CPQ * B)
                a = spool.tile([128, N_HT, IQ], fp8, tag=f"t1_{q}")
                bq = spool.tile([128, N_HT, IQ], fp8, tag=f"t2_{q}")
                wq = spool.tile([128, CPQ * H_LOC], f16, tag=f"w_{q}")
                xq = spool.tile([128, CPQ * B], fp8, tag=f"x_{q}")
                # split issue across SP and Act queues: each DMACopy holds its
                # sequencer through the (serial) HWDGE stage, so one queue
                # alone throttles issue to ~1.5us/op
                nc.sync.dma_start(a[:], t1_v[:, :, isl])
                nc.sync.dma_start(bq[:], t2_v[:, :, isl])
                nc.sync.dma_start(wq[:], w_d[:, wsl])
                # late-consumed x transfers go via Pool's SWDGE: bypasses the
                # serial HWDGE stage and the (saturated) SP sequencer
                if q == 0:
                    nc.sync.dma_start(xq[:], x_d[:, xsl])
                else:
                    nc.gpsimd.dma_start(xq[:], x_d[:, xsl])
                t1_8.append(a); t2_8.append(bq); w16.append(wq); x8.append(xq)

            # ---- PE warmup: keep HAM at full clock until real matmuls ----
            ident16 = cpool.tile([128, 128], f16)
            make_identity(nc, ident16)
            warm = psum_w.tile([128, 128], f32, name="warm")

            def fill_pe(n):
                for _ in range(n):
                    nc.tensor.matmul(warm[:], ident16[:], ident16[:],
                                     start=True, stop=True, skip_group_check=True)

            fill_pe(N_WARM)

            # ---- per-h params -> bilinear coeffs (scaled), thresholds ----
            coef = []   # (A, B, C, D) [128,1] f32 APs per ht
            thr = []
            for ht in range(N_HT):
                p0 = 6 * ht
                sig = cpool.tile([128, 4], f32, tag=f"sig{ht}")
                nc.scalar.activation(sig[:], par[:, p0:p0 + 4], AF.Sigmoid)
                oma = cpool.tile([128, 4], f32, tag=f"oma{ht}")
                # SCALE * 0.5 * (1 - sig)
                nc.vector.tensor_scalar(oma[:], sig[:], -0.5 * SCALE, 0.5 * SCALE,
                                        op0=ALU.mult, op1=ALU.add)
                c = cpool.tile([128, 4], f32, tag=f"cf{ht}")
                o = lambda k: oma[:, k:k + 1]
                nc.vector.tensor_copy(c[:, 0:1], o(0))                        # A
                nc.vector.tensor_tensor(c[:, 1:2], o(1), o(0), ALU.subtract)  # B
                nc.vector.tensor_tensor(c[:, 2:3], o(2), o(0), ALU.subtract)  # C
                t = cpool.tile([128, 1], f32, tag=f"cft{ht}")
                nc.vector.tensor_tensor(t[:], o(3), o(2), ALU.subtract)
                nc.vector.tensor_tensor(c[:, 3:4], t[:], c[:, 1:2], ALU.subtract)  # D
                coef.append((c[:, 0:1], c[:, 1:2], c[:, 2:3], c[:, 3:4]))

                sigm = cpool.tile([128, 1], f32, tag=f"sm{ht}")
                nc.scalar.activation(sigm[:], par[:, p0 + 4:p0 + 5], AF.Sigmoid)
                omb = cpool.tile([128, 1], f32, tag=f"ob{ht}")
                nc.vector.tensor_scalar(omb[:], sigm[:], -1.0, 1.0,
                                        op0=ALU.mult, op1=ALU.add)
                rcp = cpool.tile([128, 1], f32, tag=f"rc{ht}")
                nc.vector.reciprocal(rcp[:], omb[:])
                tb = cpool.tile([128, 1], f32, tag=f"tb{ht}")
                nc.vector.tensor_scalar(tb[:], par[:, p0 + 5:p0 + 6], SCALE, None,
                                        op0=ALU.mult)
                th = cpool.tile([128, 1], f32, tag=f"th{ht}")
                nc.vector.scalar_tensor_tensor(th[:], rcp[:], SCALE, tb[:],
                                               ALU.mult, ALU.subtract)
                thr.append(th)

            po = [psum_o.tile([128, B], f32, tag="po", name=f"po{ht}")
                  for ht in range(N_HT)]

            # ---- streamed build + transpose + matmuls ----
            for q in range(NQ):
                t1f = spool.tile([128, N_HT, IQ], f16, tag=f"t1f_{q}")
                nc.scalar.copy(t1f[:], t1_8[q][:])          # Act cvt (both ht)
                t2f = spool.tile([128, N_HT, IQ], f16, tag=f"t2f_{q}")
                nc.gpsimd.tensor_copy(t2f[:], t2_8[q][:])   # Pool cvt
                # w view: [p, c, ht, h]
                wv = w16[q][:].rearrange("p (c t h) -> p c t h", c=CPQ, t=N_HT)

                # build both h-tiles first so the (PSUM-gated) readbacks don't
                # block the in-order DVE queue
                Ps = []
                for ht in range(N_HT):
                    A, Bc, Cc, D = coef[ht]
                    t1h = t1f[:, ht, :]
                    Q = spool.tile([128, IQ], f16, tag=f"Q_{q}{ht}")
                    nc.vector.tensor_scalar(Q[:], t1h, D, Cc,
                                            op0=ALU.mult, op1=ALU.add)
                    nc.vector.tensor_tensor(Q[:], Q[:], t2f[:, ht, :], ALU.mult)
                    P = spool.tile([128, IQ], f16, tag=f"P_{q}{ht}")
                    nc.vector.tensor_scalar(P[:], t1h, Bc, A,
                                            op0=ALU.mult, op1=ALU.add)
                    nc.vector.tensor_tensor(P[:], P[:], Q[:], ALU.add)
                    Ps.append(P)

                for ht in range(N_HT):
                    pt = psum_t.tile([128, IQ], f16, tag="pt", name=f"pt{q}_{ht}")
                    for c in range(CPQ):
                        cs = slice(c * 128, (c + 1) * 128)
                        nc.tensor.transpose(pt[:, cs], Ps[ht][:, cs], ident16[:])
                    # PSUM readback fused with +W (DVE; GPSIMD can't read PSUM)
                    wc = spool.tile([128, IQ], f16, tag=f"wc_{q}{ht}")
                    nc.vector.tensor_tensor(
                        wc[:].rearrange("p (c h) -> p c h", c=CPQ),
                        pt[:].rearrange("p (c h) -> p c h", c=CPQ),
                        wv[:, :, ht, :], ALU.add)

                    for c in range(CPQ):
                        gc = q * CPQ + c
                        nc.tensor.matmul(
                            po[ht][:],
                            wc[:, c * 128:(c + 1) * 128],
                            x8[q][:, c * B:(c + 1) * B],
                            start=(gc == 0), stop=(gc == N_CHUNK - 1),
                            skip_group_check=True,
                        )
                    fill_pe(N_FILL)

            # ---- threshold + store ----
            for ht in range(N_HT):
                res = cpool.tile([128, B], fp8, tag=f"res{ht}")
                nc.vector.tensor_scalar(res[:], po[ht][:], thr[ht][:], None,
                                        op0=ALU.is_ge)
                nc.sync.dma_start(out_d[ht * 128:(ht + 1) * 128, :], res[:])

    nc.compile()
    return nc


def _get_nc(reps=1):
    key = "nc"
    if key not in _CACHED:
        _CACHED[key] = _build_bass()
    return _CACHED[key]


def _pack_hmaj(arr):
    """[H_LOC, I] -> [128, N_HT*I] h-major (partition = h%128, ht blocks)."""
    return np.ascontiguousarray(
        arr.reshape(N_HT, 128, I).transpose(1, 0, 2).reshape(128, N_HT * I)
    )


def _pack_imaj(arr_T, width):
    """[I, width] i-major -> [128, N_CHUNK*width], col block c = k-chunk c."""
    return np.ascontiguousarray(
        arr_T.reshape(N_CHUNK, 128, width).transpose(1, 0, 2)
        .reshape(128, N_CHUNK * width)
    )


def kernel(**inputs):
    global LAST_RESULTS
    from concourse.bass_utils import run_bass_kernel_spmd

    x = np.asarray(inputs["x"], dtype=np.float32)
    W = np.asarray(inputs["W"], dtype=np.float32)
    b = np.asarray(inputs["b"], dtype=np.float32)
    tau_m = np.asarray(inputs["tau_m"], dtype=np.float32)
    tau_n = np.asarray(inputs["tau_n"], dtype=np.float32)
    mask = np.asarray(inputs["mask"], dtype=np.float32)

    fp8 = ml_dtypes.float8_e4m3
    idx = (mask[:, :, 1] + 2.0 * mask[:, :, 2] + 3.0 * mask[:, :, 3]).astype(np.int8)
    t1 = (idx & 1).astype(np.float32)      # [H, I]
    t2 = (idx >> 1).astype(np.float32)
    xp = _pack_imaj(np.ascontiguousarray(x.T), B).astype(fp8)

    nc = _get_nc()
    in_maps = []
    for c in range(NCORES):
        hs = slice(c * H_LOC, (c + 1) * H_LOC)
        par = np.zeros((128, 6 * N_HT), dtype=np.float32)
        for ht in range(N_HT):
            hh = slice(c * H_LOC + ht * 128, c * H_LOC + (ht + 1) * 128)
            par[:, 6 * ht:6 * ht + 4] = tau_n[hh]
            par[:, 6 * ht + 4] = tau_m[hh]
            par[:, 6 * ht + 5] = b[hh]
        # w layout: [128(i in chunk), (chunk, ht, h)]
        wT = np.ascontiguousarray(W[hs].T) * np.float32(SCALE)   # [I, 256]
        wp = wT.reshape(N_CHUNK, 128, N_HT, 128).transpose(1, 0, 2, 3) \
               .reshape(128, N_CHUNK * H_LOC)
        in_maps.append({
            "t1": _pack_hmaj(t1[hs]).astype(fp8),
            "t2": _pack_hmaj(t2[hs]).astype(fp8),
            "w": np.ascontiguousarray(wp).astype(np.float16),
            "x": xp,
            "par": par,
        })

    try:
        res = run_bass_kernel_spmd(
            nc, in_maps, core_ids=list(range(NCORES)), trace=TRACE,
        )
    except Exception:
        if not TRACE:
            raise
        res = run_bass_kernel_spmd(
            nc, in_maps, core_ids=list(range(NCORES)), trace=False,
        )
    LAST_RESULTS = res
    outT = np.concatenate([r["out"].astype(np.float32) for r in res.results], axis=0)
    return np.ascontiguousarray(outT.T)                               # [B, H]


# revision 16
# speedup vs baseline: 1.1728x; 1.1728x over previous
"""Trainium2 Bass kernel for the DH-LIF node single-step forward.

Math: the mask is one-hot over the branch dim NB, so

    spike = ( x @ (W + M).T + b >= 1/(1-beta) ),
    M[h,i] = oma[h, idx[h,i]],  oma[h,k] = 0.5*(1 - sigmoid(tau_n[h,k]))

The 2-bit branch index idx is re-encoded on the host (losslessly) as two
{0,1} bit planes t1 = idx&1, t2 = idx>>1 (fp8, h-major).  On device, M is
built per h-tile in the bilinear-bit basis with per-partition coefficients

    M = (A + B*t1) + (C + D*t1)*t2        (A..D per-h from sigmoid(tau_n))

as two DVE tensor_scalar ops (4x mode, dual per-partition scalars) and two
tensor_tensor ops (2x); multiplies by {0,1} are exact in fp16, so precision
matches a single-rounding f32 build.  Everything is scaled by 256 to keep
fp16 away from subnormals.  The [h,i] result is flipped to i-major with PE
transposes; the PSUM->SBUF readback is fused with the +W pass (gpsimd
tensor_tensor against host-pre-transposed fp16 W), giving the matmul lhsT.
x ships transposed fp8 (spikes exact in fp8); one fp16 x fp8 matmul pass
accumulates out[h,b] over 32 k-chunks; PSUM is thresholded against
256/(1-beta) - 256*b per partition.  Dummy PE matmuls pad pipeline gaps so
the tensor engine's activity-gated clock stays at full rate.

Sharding: hidden dim split across 8 cores (h_loc = 256); x replicated.
Host does layout/dtype prep (bit-plane extraction, transposes, packing)
and the final gather/cast.
"""

import numpy as np
import ml_dtypes

B, I, H, NB = 512, 4096, 2048, 4
NCORES = 8
H_LOC = H // NCORES          # 256
N_HT = H_LOC // 128          # 2 h-tiles of 128
N_CHUNK = I // 128           # 32 matmul k-chunks
NQ = 4                       # DMA/compute super-chunks
CPQ = N_CHUNK // NQ          # 8 k-chunks per super-chunk
IQ = CPQ * 128               # 1024 i per super-chunk
SCALE = 256.0                # fp16 subnormal guard (power of 2, lossless)
N_WARM = 20                  # initial PE warmup matmuls
N_FILL = 18                  # dummy matmuls per (q,ht) to keep PE clock hot

TRACE = False
LAST_RESULTS = None
_CACHED = {}


def _build_bass():
    import concourse.bacc as bacc
    import concourse.mybir as mybir
    from concourse.tile import TileContext
    from concourse.masks import make_identity

    f32 = mybir.dt.float32
    f16 = mybir.dt.float16
    fp8 = mybir.dt.float8e4
    AF = mybir.ActivationFunctionType
    ALU = mybir.AluOpType

    nc = bacc.Bacc("TRN2", target_bir_lowering=False, debug=False)

    # h-major bit planes: [128, (ht, i)]
    t1_d = nc.dram_tensor("t1", [128, N_HT * I], fp8, kind="ExternalInput")
    t2_d = nc.dram_tensor("t2", [128, N_HT * I], fp8, kind="ExternalInput")
    # i-major transposed W * SCALE: [128, (chunk, ht, h)]
    w_d = nc.dram_tensor("w", [128, N_CHUNK * H_LOC], f16, kind="ExternalInput")
    # i-major x: [128, (chunk, b)]
    x_d = nc.dram_tensor("x", [128, N_CHUNK * B], fp8, kind="ExternalInput")
    par_d = nc.dram_tensor("par", [128, 6 * N_HT], f32, kind="ExternalInput")
    out_d = nc.dram_tensor("out", [H_LOC, B], fp8, kind="ExternalOutput")

    t1_v = t1_d.rearrange("p (t i) -> p t i", t=N_HT)
    t2_v = t2_d.rearrange("p (t i) -> p t i", t=N_HT)

    with TileContext(nc) as tc:
        with (
            tc.tile_pool(name="const", bufs=1) as cpool,
            tc.tile_pool(name="strm", bufs=1) as spool,
            tc.tile_pool(name="po", bufs=2, space="PSUM") as psum_o,
            tc.tile_pool(name="pt", bufs=3, space="PSUM") as psum_t,
            tc.tile_pool(name="pw", bufs=1, space="PSUM") as psum_w,
        ):
            # ---- bulk DMAs, super-chunk interleaved, issue first ----
            par = cpool.tile([128, 6 * N_HT], f32)
            nc.sync.dma_start(par[:], par_d[:, :])
            t1_8, t2_8, w16, x8 = [], [], [], []
            for q in range(NQ):
                isl = slice(q * IQ, (q + 1) * IQ)
                wsl = slice(q * CPQ * H_LOC, (q + 1) * CPQ * H_LOC)
                xsl = slice(q * CPQ * B, (q + 1) * CPQ * B)
                a = spool.tile([128, N_HT, IQ], fp8, tag=f"t1_{q}")
                bq = spool.tile([128, N_HT, IQ], fp8, tag=f"t2_{q}")
                wq = spool.tile([128, CPQ * H_LOC], f16, tag=f"w_{q}")
                xq = spool.tile([128, CPQ * B], fp8, tag=f"x_{q}")
                # split issue across SP and Act queues: each DMACopy holds its
                # sequencer through the (serial) HWDGE stage, so one queue
                # alone throttles issue to ~1.5us/op
                nc.sync.dma_start(a[:], t1_v[:, :, isl])
                nc.sync.dma_start(bq[:], t2_v[:, :, isl])
                nc.sync.dma_start(wq[:], w_d[:, wsl])
                nc.sync.dma_start(xq[:], x_d[:, xsl])
                t1_8.append(a); t2_8.append(bq); w16.append(wq); x8.append(xq)

            # ---- PE warmup: keep HAM at full clock until real matmuls ----
            ident16 = cpool.tile([128, 128], f16)
            make_identity(nc, ident16)
            warm = psum_w.tile([128, 128], f32, name="warm")

            def fill_pe(n):
                for _ in range(n):
                    nc.tensor.matmul(warm[:], ident16[:], ident16[:],
                                     start=True, stop=True, skip_group_check=True)

            fill_pe(N_WARM)

            # ---- per-h params -> bilinear coeffs (scaled), thresholds ----
            coef = []   # (A, B, C, D) [128,1] f32 APs per ht
            thr = []
            for ht in range(N_HT):
                p0 = 6 * ht
                sig = cpool.tile([128, 4], f32, tag=f"sig{ht}")
                nc.scalar.activation(sig[:], par[:, p0:p0 + 4], AF.Sigmoid)
                oma = cpool.tile([128, 4], f32, tag=f"oma{ht}")
                # SCALE * 0.5 * (1 - sig)
                nc.vector.tensor_scalar(oma[:], sig[:], -0.5 * SCALE, 0.5 * SCALE,
                                        op0=ALU.mult, op1=ALU.add)
                c = cpool.tile([128, 4], f32, tag=f"cf{ht}")
                o = lambda k: oma[:, k:k + 1]
                nc.vector.tensor_copy(c[:, 0:1], o(0))                        # A
                nc.vector.tensor_tensor(c[:, 1:2], o(1), o(0), ALU.subtract)  # B
                nc.vector.tensor_tensor(c[:, 2:3], o(2), o(0), ALU.subtract)  # C
                t = cpool.tile([128, 1], f32, tag=f"cft{ht}")
                nc.vector.tensor_tensor(t[:], o(3), o(2), ALU.subtract)
                nc.vector.tensor_tensor(c[:, 3:4], t[:], c[:, 1:2], ALU.subtract)  # D
                coef.append((c[:, 0:1], c[:, 1:2], c[:, 2:3], c[:, 3:4]))

                sigm = cpool.tile([128, 1], f32, tag=f"sm{ht}")
                nc.scalar.activation(sigm[:], par[:, p0 + 4:p0 + 5], AF.Sigmoid)
                omb = cpool.tile([128, 1], f32, tag=f"ob{ht}")
                nc.vector.tensor_scalar(omb[:], sigm[:], -1.0, 1.0,
                                        op0=ALU.mult, op1=ALU.add)
                rcp = cpool.tile([128, 1], f32, tag=f"rc{ht}")
                nc.vector.reciprocal(rcp[:], omb[:])
                tb = cpool.tile([128, 1], f32, tag=f"tb{ht}")
                nc.vector.tensor_scalar(tb[:], par[:, p0 + 5:p0 + 6], SCALE, None,
                                        op0=ALU.mult)
                th = cpool.tile([128, 1], f32, tag=f"th{ht}")
                nc.vector.scalar_tensor_tensor(th[:], rcp[:], SCALE, tb[:],
                                               ALU.mult, ALU.subtract)
                thr.append(th)

            po = [psum_o.tile([128, B], f32, tag="po", name=f"po{ht}")
                  for ht in range(N_HT)]

            # ---- streamed build + transpose + matmuls ----
            for q in range(NQ):
                t1f = spool.tile([128, N_HT, IQ], f16, tag=f"t1f_{q}")
                nc.scalar.copy(t1f[:], t1_8[q][:])          # Act cvt (both ht)
                t2f = spool.tile([128, N_HT, IQ], f16, tag=f"t2f_{q}")
                nc.gpsimd.tensor_copy(t2f[:], t2_8[q][:])   # Pool cvt
                # w view: [p, c, ht, h]
                wv = w16[q][:].rearrange("p (c t h) -> p c t h", c=CPQ, t=N_HT)

                # build both h-tiles first so the (PSUM-gated) readbacks don't
                # block the in-order DVE queue
                Ps = []
                for ht in range(N_HT):
                    A, Bc, Cc, D = coef[ht]
                    t1h = t1f[:, ht, :]
                    Q = spool.tile([128, IQ], f16, tag=f"Q_{q}{ht}")
                    nc.vector.tensor_scalar(Q[:], t1h, D, Cc,
                                            op0=ALU.mult, op1=ALU.add)
                    nc.vector.tensor_tensor(Q[:], Q[:], t2f[:, ht, :], ALU.mult)
                    P = spool.tile([128, IQ], f16, tag=f"P_{q}{ht}")
                    nc.vector.tensor_scalar(P[:], t1h, Bc, A,
                                            op0=ALU.mult, op1=ALU.add)
                    nc.vector.tensor_tensor(P[:], P[:], Q[:], ALU.add)
                    Ps.append(P)

                for ht in range(N_HT):
                    pt = psum_t.tile([128, IQ], f16, tag="pt", name=f"pt{q}_{ht}")
                    for c in range(CPQ):
                        cs = slice(c * 128, (c + 1) * 128)
                        nc.tensor.transpose(pt[:, cs], Ps[ht][:, cs], ident16[:])
                    # PSUM readback fused with +W (DVE; GPSIMD can't read PSUM)
                    wc = spool.tile([128, IQ], f16, tag=f"wc_{q}{ht}")
                    nc.vector.tensor_tensor(
                        wc[:].rearrange("p (c h) -> p c h", c=CPQ),
                        pt[:].rearrange("p (c h) -> p c h", c=CPQ),
                        wv[:, :, ht, :], ALU.add)

                    for c in range(CPQ):
                        gc = q * CPQ + c
                        nc.tensor.matmul(
                            po[ht][:],
                            wc[:, c * 128:(c + 1) * 128],
                            x8[q][:, c * B:(c + 1) * B],
                            start=(gc == 0), stop=(gc == N_CHUNK - 1),
                            skip_group_check=True,
                        )
                    fill_pe(N_FILL)

            # ---- threshold + store ----
            for ht in range(N_HT):
                res = cpool.tile([128, B], fp8, tag=f"res{ht}")
                nc.vector.tensor_scalar(res[:], po[ht][:], thr[ht][:], None,
                                        op0=ALU.is_ge)
                nc.sync.dma_start(out_d[ht * 128:(ht + 1) * 128, :], res[:])

    nc.compile()
    return nc


def _get_nc(reps=1):
    key = "nc"
    if key not in _CACHED:
        _CACHED[key] = _build_bass()
    return _CACHED[key]


def _pack_hmaj(arr):
    """[H_LOC, I] -> [128, N_HT*I] h-major (partition = h%128, ht blocks)."""
    return np.ascontiguousarray(
        arr.reshape(N_HT, 128, I).transpose(1, 0, 2).reshape(128, N_HT * I)
    )


def _pack_imaj(arr_T, width):
    """[I, width] i-major -> [128, N_CHUNK*width], col block c = k-chunk c."""
    return np.ascontiguousarray(
        arr_T.reshape(N_CHUNK, 128, width).transpose(1, 0, 2)
        .reshape(128, N_CHUNK * width)
    )


def kernel(**inputs):
    global LAST_RESULTS
    from concourse.bass_utils import run_bass_kernel_spmd

    x = np.asarray(inputs["x"], dtype=np.float32)
    W = np.asarray(inputs["W"], dtype=np.float32)
    b = np.asarray(inputs["b"], dtype=np.float32)
    tau_m = np.asarray(inputs["tau_m"], dtype=np.float32)
    tau_n = np.asarray(inputs["tau_n"], dtype=np.float32)
    mask = np.asarray(inputs["mask"], dtype=np.float32)

    fp8 = ml_dtypes.float8_e4m3
    idx = (mask[:, :, 1] + 2.0 * mask[:, :, 2] + 3.0 * mask[:, :, 3]).astype(np.int8)
    t1 = (idx & 1).astype(np.float32)      # [H, I]
    t2 = (idx >> 1).astype(np.float32)
    xp = _pack_imaj(np.ascontiguousarray(x.T), B).astype(fp8)

    nc = _get_nc()
    in_maps = []
    for c in range(NCORES):
        hs = slice(c * H_LOC, (c + 1) * H_LOC)
        par = np.zeros((128, 6 * N_HT), dtype=np.float32)
        for ht in range(N_HT):
            hh = slice(c * H_LOC + ht * 128, c * H_LOC + (ht + 1) * 128)
            par[:, 6 * ht:6 * ht + 4] = tau_n[hh]
            par[:, 6 * ht + 4] = tau_m[hh]
            par[:, 6 * ht + 5] = b[hh]
        # w layout: [128(i in chunk), (chunk, ht, h)]
        wT = np.ascontiguousarray(W[hs].T) * np.float32(SCALE)   # [I, 256]
        wp = wT.reshape(N_CHUNK, 128, N_HT, 128).transpose(1, 0, 2, 3) \
               .reshape(128, N_CHUNK * H_LOC)
        in_maps.append({
            "t1": _pack_hmaj(t1[hs]).astype(fp8),
            "t2": _pack_hmaj(t2[hs]).astype(fp8),
            "w": np.ascontiguousarray(wp).astype(np.float16),
            "x": xp,
            "par": par,
        })

    try:
        res = run_bass_kernel_spmd(
            nc, in_maps, core_ids=list(range(NCORES)), trace=TRACE,
        )
    except Exception:
        if not TRACE:
            raise
        res = run_bass_kernel_spmd(
            nc, in_maps, core_ids=list(range(NCORES)), trace=False,
        )
    LAST_RESULTS = res
    outT = np.concatenate([r["out"].astype(np.float32) for r in res.results], axis=0)
    return np.ascontiguousarray(outT.T)                               # [B, H]


# revision 22
# speedup vs baseline: 1.1864x; 1.0116x over previous
"""Trainium2 Bass kernel for the DH-LIF node single-step forward.

Math: the mask is one-hot over the branch dim NB, so

    spike = ( x @ (W + M).T + b >= 1/(1-beta) ),
    M[h,i] = oma[h, idx[h,i]],  oma[h,k] = 0.5*(1 - sigmoid(tau_n[h,k]))

The 2-bit branch index idx is re-encoded on the host (losslessly) as two
{0,1} bit planes t1 = idx&1, t2 = idx>>1 (fp8, h-major).  On device, M is
built per h-tile in the bilinear-bit basis with per-partition coefficients

    M = (A + B*t1) + (C + D*t1)*t2        (A..D per-h from sigmoid(tau_n))

as two DVE tensor_scalar ops (4x mode, dual per-partition scalars) and two
tensor_tensor ops (2x); multiplies by {0,1} are exact in fp16, so precision
matches a single-rounding f32 build.  Everything is scaled by 256 to keep
fp16 away from subnormals.  The [h,i] result is flipped to i-major with PE
transposes; the PSUM->SBUF readback is fused with the +W pass (gpsimd
tensor_tensor against host-pre-transposed fp16 W), giving the matmul lhsT.
x ships transposed fp8 (spikes exact in fp8); one fp16 x fp8 matmul pass
accumulates out[h,b] over 32 k-chunks; PSUM is thresholded against
256/(1-beta) - 256*b per partition.  Dummy PE matmuls pad pipeline gaps so
the tensor engine's activity-gated clock stays at full rate.

Sharding: hidden dim split across 8 cores (h_loc = 256); x replicated.
Host does layout/dtype prep (bit-plane extraction, transposes, packing)
and the final gather/cast.
"""

import numpy as np
import ml_dtypes

B, I, H, NB = 512, 4096, 2048, 4
NCORES = 8
H_LOC = H // NCORES          # 256
N_HT = H_LOC // 128          # 2 h-tiles of 128
N_CHUNK = I // 128           # 32 matmul k-chunks
NQ = 4                       # DMA/compute super-chunks
CPQ = N_CHUNK // NQ          # 8 k-chunks per super-chunk
IQ = CPQ * 128               # 1024 i per super-chunk
SCALE = 256.0                # fp16 subnormal guard (power of 2, lossless)
N_WARM = 20                  # initial PE warmup matmuls
N_FILL = 18                  # dummy matmuls per (q,ht) to keep PE clock hot

TRACE = False
LAST_RESULTS = None
_CACHED = {}


def _build_bass():
    import concourse.bacc as bacc
    import concourse.mybir as mybir
    from concourse.tile import TileContext
    from concourse.masks import make_identity

    f32 = mybir.dt.float32
    f16 = mybir.dt.float16
    fp8 = mybir.dt.float8e4
    AF = mybir.ActivationFunctionType
    ALU = mybir.AluOpType

    nc = bacc.Bacc("TRN2", target_bir_lowering=False, debug=False)

    # h-major bit planes, merged per super-chunk: [128, (q, plane, ht, i_q)]
    tb_d = nc.dram_tensor("tb", [128, NQ * 2 * N_HT * IQ], fp8,
                          kind="ExternalInput")
    # i-major transposed W * SCALE: [128, (chunk, ht, h)]
    w_d = nc.dram_tensor("w", [128, N_CHUNK * H_LOC], f16, kind="ExternalInput")
    # i-major x: [128, (chunk, b)]
    x_d = nc.dram_tensor("x", [128, N_CHUNK * B], fp8, kind="ExternalInput")
    par_d = nc.dram_tensor("par", [128, 6 * N_HT], f32, kind="ExternalInput")
    out_d = nc.dram_tensor("out", [H_LOC, B], fp8, kind="ExternalOutput")

    tb_v = tb_d.rearrange("p (q l t i) -> p q l t i", q=NQ, l=2, t=N_HT)

    with TileContext(nc) as tc:
        with (
            tc.tile_pool(name="const", bufs=1) as cpool,
            tc.tile_pool(name="strm", bufs=1) as spool,
            tc.tile_pool(name="po", bufs=2, space="PSUM") as psum_o,
            tc.tile_pool(name="pt", bufs=3, space="PSUM") as psum_t,
            tc.tile_pool(name="pw", bufs=1, space="PSUM") as psum_w,
        ):
            # ---- bulk DMAs, super-chunk interleaved, issue first ----
            par = cpool.tile([128, 6 * N_HT], f32)
            nc.sync.dma_start(par[:], par_d[:, :])
            t1_8, w16, x8 = [], [], []
            for q in range(NQ):
                isl = slice(q * IQ, (q + 1) * IQ)
                wsl = slice(q * CPQ * H_LOC, (q + 1) * CPQ * H_LOC)
                xsl = slice(q * CPQ * B, (q + 1) * CPQ * B)
                tbq = spool.tile([128, 2, N_HT, IQ], fp8, tag=f"tb_{q}")
                wq = spool.tile([128, CPQ * H_LOC], f16, tag=f"w_{q}")
                xq = spool.tile([128, CPQ * B], fp8, tag=f"x_{q}")
                nc.sync.dma_start(tbq[:], tb_v[:, q])
                nc.sync.dma_start(wq[:], w_d[:, wsl])
                nc.sync.dma_start(xq[:], x_d[:, xsl])
                t1_8.append(tbq); w16.append(wq); x8.append(xq)

            # ---- PE warmup: keep HAM at full clock until real matmuls ----
            ident16 = cpool.tile([128, 128], f16)
            make_identity(nc, ident16)
            warm = psum_w.tile([128, 128], f32, name="warm")

            def fill_pe(n):
                for _ in range(n):
                    nc.tensor.matmul(warm[:], ident16[:], ident16[:],
                                     start=True, stop=True, skip_group_check=True)

            fill_pe(N_WARM)

            # ---- per-h params -> bilinear coeffs (scaled), thresholds ----
            coef = []   # (A, B, C, D) [128,1] f32 APs per ht
            thr = []
            for ht in range(N_HT):
                p0 = 6 * ht
                sig = cpool.tile([128, 4], f32, tag=f"sig{ht}")
                nc.scalar.activation(sig[:], par[:, p0:p0 + 4], AF.Sigmoid)
                oma = cpool.tile([128, 4], f32, tag=f"oma{ht}")
                # SCALE * 0.5 * (1 - sig)
                nc.vector.tensor_scalar(oma[:], sig[:], -0.5 * SCALE, 0.5 * SCALE,
                                        op0=ALU.mult, op1=ALU.add)
                c = cpool.tile([128, 4], f32, tag=f"cf{ht}")
                o = lambda k: oma[:, k:k + 1]
                nc.vector.tensor_copy(c[:, 0:1], o(0))                        # A
                nc.vector.tensor_tensor(c[:, 1:2], o(1), o(0), ALU.subtract)  # B
                nc.vector.tensor_tensor(c[:, 2:3], o(2), o(0), ALU.subtract)  # C
                t = cpool.tile([128, 1], f32, tag=f"cft{ht}")
                nc.vector.tensor_tensor(t[:], o(3), o(2), ALU.subtract)
                nc.vector.tensor_tensor(c[:, 3:4], t[:], c[:, 1:2], ALU.subtract)  # D
                coef.append((c[:, 0:1], c[:, 1:2], c[:, 2:3], c[:, 3:4]))

                sigm = cpool.tile([128, 1], f32, tag=f"sm{ht}")
                nc.scalar.activation(sigm[:], par[:, p0 + 4:p0 + 5], AF.Sigmoid)
                omb = cpool.tile([128, 1], f32, tag=f"ob{ht}")
                nc.vector.tensor_scalar(omb[:], sigm[:], -1.0, 1.0,
                                        op0=ALU.mult, op1=ALU.add)
                rcp = cpool.tile([128, 1], f32, tag=f"rc{ht}")
                nc.vector.reciprocal(rcp[:], omb[:])
                tb = cpool.tile([128, 1], f32, tag=f"tb{ht}")
                nc.vector.tensor_scalar(tb[:], par[:, p0 + 5:p0 + 6], SCALE, None,
                                        op0=ALU.mult)
                th = cpool.tile([128, 1], f32, tag=f"th{ht}")
                nc.vector.scalar_tensor_tensor(th[:], rcp[:], SCALE, tb[:],
                                               ALU.mult, ALU.subtract)
                thr.append(th)

            po = [psum_o.tile([128, B], f32, tag="po", name=f"po{ht}")
                  for ht in range(N_HT)]

            # ---- streamed build + transpose + matmuls ----
            for q in range(NQ):
                t1f = spool.tile([128, N_HT, IQ], f16, tag=f"t1f_{q}")
                nc.scalar.copy(t1f[:], t1_8[q][:, 0])       # Act cvt (both ht)
                t2f = spool.tile([128, N_HT, IQ], f16, tag=f"t2f_{q}")
                nc.gpsimd.tensor_copy(t2f[:], t1_8[q][:, 1])  # Pool cvt
                # w view: [p, c, ht, h]
                wv = w16[q][:].rearrange("p (c t h) -> p c t h", c=CPQ, t=N_HT)

                # build both h-tiles first so the (PSUM-gated) readbacks don't
                # block the in-order DVE queue
                Ps = []
                for ht in range(N_HT):
                    A, Bc, Cc, D = coef[ht]
                    t1h = t1f[:, ht, :]
                    Q = spool.tile([128, IQ], f16, tag=f"Q_{q}{ht}")
                    nc.vector.tensor_scalar(Q[:], t1h, D, Cc,
                                            op0=ALU.mult, op1=ALU.add)
                    nc.vector.tensor_tensor(Q[:], Q[:], t2f[:, ht, :], ALU.mult)
                    P = spool.tile([128, IQ], f16, tag=f"P_{q}{ht}")
                    nc.vector.tensor_scalar(P[:], t1h, Bc, A,
                                            op0=ALU.mult, op1=ALU.add)
                    nc.vector.tensor_tensor(P[:], P[:], Q[:], ALU.add)
                    Ps.append(P)

                for ht in range(N_HT):
                    pt = psum_t.tile([128, IQ], f16, tag="pt", name=f"pt{q}_{ht}")
                    for c in range(CPQ):
                        cs = slice(c * 128, (c + 1) * 128)
                        nc.tensor.transpose(pt[:, cs], Ps[ht][:, cs], ident16[:])
                    # PSUM readback fused with +W (DVE; GPSIMD can't read PSUM)
                    wc = spool.tile([128, IQ], f16, tag=f"wc_{q}{ht}")
                    nc.vector.tensor_tensor(
                        wc[:].rearrange("p (c h) -> p c h", c=CPQ),
                        pt[:].rearrange("p (c h) -> p c h", c=CPQ),
                        wv[:, :, ht, :], ALU.add)

                    for c in range(CPQ):
                        gc = q * CPQ + c
                        nc.tensor.matmul(
                            po[ht][:],
                            wc[:, c * 128:(c + 1) * 128],
                            x8[q][:, c * B:(c + 1) * B],
                            start=(gc == 0), stop=(gc == N_CHUNK - 1),
                            skip_group_check=True,
                        )
                    fill_pe(N_FILL)

            # ---- threshold + store ----
            for ht in range(N_HT):
                res = cpool.tile([128, B], fp8, tag=f"res{ht}")
                nc.vector.tensor_scalar(res[:], po[ht][:], thr[ht][:], None,
                                        op0=ALU.is_ge)
                nc.sync.dma_start(out_d[ht * 128:(ht + 1) * 128, :], res[:])

    nc.compile()
    return nc


def _get_nc(reps=1):
    key = "nc"
    if key not in _CACHED:
        _CACHED[key] = _build_bass()
    return _CACHED[key]


def _pack_hmaj(arr):
    """[H_LOC, I] -> [128, N_HT*I] h-major (partition = h%128, ht blocks)."""
    return np.ascontiguousarray(
        arr.reshape(N_HT, 128, I).transpose(1, 0, 2).reshape(128, N_HT * I)
    )


def _pack_imaj(arr_T, width):
    """[I, width] i-major -> [128, N_CHUNK*width], col block c = k-chunk c."""
    return np.ascontiguousarray(
        arr_T.reshape(N_CHUNK, 128, width).transpose(1, 0, 2)
        .reshape(128, N_CHUNK * width)
    )


def kernel(**inputs):
    global LAST_RESULTS
    from concourse.bass_utils import run_bass_kernel_spmd

    x = np.asarray(inputs["x"], dtype=np.float32)
    W = np.asarray(inputs["W"], dtype=np.float32)
    b = np.asarray(inputs["b"], dtype=np.float32)
    tau_m = np.asarray(inputs["tau_m"], dtype=np.float32)
    tau_n = np.asarray(inputs["tau_n"], dtype=np.float32)
    mask = np.asarray(inputs["mask"], dtype=np.float32)

    fp8 = ml_dtypes.float8_e4m3
    idx = (mask[:, :, 1] + 2.0 * mask[:, :, 2] + 3.0 * mask[:, :, 3]).astype(np.int8)
    t1 = (idx & 1).astype(np.float32)      # [H, I]
    t2 = (idx >> 1).astype(np.float32)
    xp = _pack_imaj(np.ascontiguousarray(x.T), B).astype(fp8)

    nc = _get_nc()
    in_maps = []
    for c in range(NCORES):
        hs = slice(c * H_LOC, (c + 1) * H_LOC)
        par = np.zeros((128, 6 * N_HT), dtype=np.float32)
        for ht in range(N_HT):
            hh = slice(c * H_LOC + ht * 128, c * H_LOC + (ht + 1) * 128)
            par[:, 6 * ht:6 * ht + 4] = tau_n[hh]
            par[:, 6 * ht + 4] = tau_m[hh]
            par[:, 6 * ht + 5] = b[hh]
        # w layout: [128(i in chunk), (chunk, ht, h)]
        wT = np.ascontiguousarray(W[hs].T) * np.float32(SCALE)   # [I, 256]
        wp = wT.reshape(N_CHUNK, 128, N_HT, 128).transpose(1, 0, 2, 3) \
               .reshape(128, N_CHUNK * H_LOC)
        # merged bit planes: [128, q, plane, ht, i_q]
        t1r = _pack_hmaj(t1[hs]).reshape(128, N_HT, NQ, IQ)
        t2r = _pack_hmaj(t2[hs]).reshape(128, N_HT, NQ, IQ)
        tb = np.stack([t1r, t2r], axis=2).transpose(0, 3, 2, 1, 4)  # p,q,l,t,i
        in_maps.append({
            "tb": np.ascontiguousarray(tb.reshape(128, -1)).astype(fp8),
            "w": np.ascontiguousarray(wp).astype(np.float16),
            "x": xp,
            "par": par,
        })

    try:
        res = run_bass_kernel_spmd(
            nc, in_maps, core_ids=list(range(NCORES)), trace=TRACE,
        )
    except Exception:
        if not TRACE:
            raise
        res = run_bass_kernel_spmd(
            nc, in_maps, core_ids=list(range(NCORES)), trace=False,
        )
    LAST_RESULTS = res
    outT = np.concatenate([r["out"].astype(np.float32) for r in res.results], axis=0)
    return np.ascontiguousarray(outT.T)                               # [B, H]
